# revision 80
# baseline (speedup 1.0000x reference)
"""Trainium2 Bass kernel for a 2-layer GAT (nn_GAT_70909910057105).

Strategy (8 NeuronCores, SPMD):
  - Core k owns target nodes [128k, 128k+128). Edges bucketed by trg//128 on
    the host (integer-only preprocessing + dtype casts).
  - Every core builds the FULL layer-1 node table locally (bf16 matmuls from
    a replicated bf16 xT) -- no collective before layer 1. Layer 2 rebuilds
    the table from the core-local x1 shard and AllGathers it.
  - A bf16 DRAM node table holds per-node rows
    [h bf16 x1024 (b,d,h layout) | a_src f32 x16 (bitcast) | pad] (1152 bf16).
    Per-edge source rows are fetched with pipelined dma_gather
    (prepare_only + trigger_dma) so gpsimd only does descriptor generation.
  - Edge features are gathered ON HOST (pure integer indexing + bf16 cast)
    and shipped pre-transposed, so pe = eT.T @ wesum is a plain bf16 matmul.
  - segment_sum is a PSUM-accumulated bf16 matmul with host-built one-hot
    masks; per-edge target alphas come from maskT.T @ [at_hi | at_res].
  - Table h layout (b,d,h) makes the per-edge exp-score broadcast multiply
    hit the DVE 2x fast mode (innermost dim packed, all operands bf16).
"""
import sys

for _p in ("/opt/trn_rl_repo", "/root/.axon_site/_ro/trn_rl_repo"):
    if _p not in sys.path:
        sys.path.insert(0, _p)

import numpy as np
import ml_dtypes
import concourse.bass as bass
import concourse.bacc as bacc
import concourse.tile as tile
from concourse import mybir
from concourse.bass_utils import run_bass_kernel_spmd
from concourse.masks import make_identity

F32 = mybir.dt.float32
BF16 = mybir.dt.bfloat16
I16 = mybir.dt.int16
NPBF = ml_dtypes.bfloat16
NPF8 = ml_dtypes.float8_e4m3
F8 = mybir.dt.float8e4

USE_PREP = False  # pipelined prepare_only gathers (see edge_loop)

N, B, C, H, D = 1024, 4, 256, 4, 64
E = 32768
NC = 8
TPC = N // NC           # target nodes per core = 128
# row: 4 x [256 h (d,h layout) | 8 (4 f32 a_src, bitcast)] then pad -> 1152
ROW = 1152
BROW = 264              # bf16 elems per b-block: 256 h + 8 a_src
NB = N * B              # 4096 (node, batch) rows
NB_LOCAL = TPC * B      # 512 local (node, batch) rows

# column permutation: j = d*H + h  ->  c = h*D + d  (h block layout (d, h))
_DH_PERM = np.array([(j % H) * D + j // H for j in range(C)], dtype=np.int64)


# --------------------------------------------------------------------------
# host-side preprocessing (integer indexing / layout / dtype casts only)
# --------------------------------------------------------------------------

def _pack_idx(vals: np.ndarray) -> np.ndarray:
    n = vals.shape[0]
    assert n % 16 == 0
    blk = vals.astype(np.int16).reshape(n // 16, 16).T
    return np.ascontiguousarray(np.tile(blk, (8, 1)))


def _sb3(w: np.ndarray, inner: int, dt=NPBF) -> np.ndarray:
    return np.ascontiguousarray(
        w.reshape(2, 128, inner).transpose(1, 0, 2).astype(dt))


def _prep(x, edge_features, src_idx, trg_idx,
          Wn1, We1, a_src1, a_tgt1, a_edge1,
          Wn2, We2, a_src2, a_tgt2, a_edge2):
    src = np.asarray(src_idx).astype(np.int64)
    trg = np.asarray(trg_idx).astype(np.int64)
    x = np.asarray(x, dtype=np.float32)

    per_core = []
    emax = 0
    for k in range(NC):
        eids = np.nonzero((trg // TPC) == k)[0]
        eids = eids[np.argsort(src[eids], kind="stable")]
        per_core.append(eids)
        emax = max(emax, len(eids))
    E_pad = ((emax + 127) // 128) * 128
    n_super = (E_pad + 511) // 512
    n_chunks = E_pad // 128

    xf = x.reshape(NB, C)

    def build_w(Wn):
        # Wn.T with columns permuted to (d,h) order -> [C, 256]
        return _sb3(np.ascontiguousarray(
            np.asarray(Wn, np.float32).T[:, _DH_PERM]), C)

    def build_ablk(a_s, a_t):
        # block-diagonal [a_src | a_tgt] -> [C, 2H]
        m = np.zeros((C, 2 * H), np.float32)
        a_s = np.asarray(a_s, np.float32)
        a_t = np.asarray(a_t, np.float32)
        for h in range(H):
            m[h * D:(h + 1) * D, h] = a_s[h]
            m[h * D:(h + 1) * D, H + h] = a_t[h]
        return _sb3(m, 2 * H)

    def build_hselb(a_e):
        # b-replicated head selector: [C, 16], col = b*H + h
        m = np.zeros((C, B * H), np.float32)
        a_e = np.asarray(a_e, np.float32)
        for h in range(H):
            for b in range(B):
                m[h * D:(h + 1) * D, b * H + h] = a_e[h]
        return _sb3(m, B * H)

    zpad8 = np.zeros((128, 2, 8), NPBF)
    wpack = np.concatenate([
        build_w(Wn1), zpad8, build_w(Wn2), zpad8,
        _sb3(np.asarray(Wn1, np.float32), C),
        _sb3(np.asarray(Wn2, np.float32), C),
        _sb3(np.asarray(We1, np.float32), C),
        _sb3(np.asarray(We2, np.float32), C),
        build_hselb(a_edge1), build_hselb(a_edge2),
        build_ablk(a_src1, a_tgt1), build_ablk(a_src2, a_tgt2),
    ], axis=2)
    common = {
        "wpack": np.ascontiguousarray(wpack),
        "ident": np.eye(128, dtype=NPBF),
        "ident32": np.eye(B * H, dtype=np.float32),
        "xT_full": _sb3(np.ascontiguousarray(xf.T), NB),
    }

    # per-superstep table prefix needed by the (src-sorted) gathers
    need = [1] * n_super
    ef = edge_features  # only sliced rows are materialized below
    in_maps = []
    for k in range(NC):
        eids = per_core[k]
        ne = len(eids)
        src_s = np.zeros(E_pad, np.int64)
        src_s[:ne] = src[eids]
        for s in range(n_super):
            hi = min((s + 1) * 512, E_pad)
            rmax = int(src_s[:hi].max()) + 1
            need[s] = max(need[s], (rmax + 31) // 32)
        mask = np.zeros((128, E_pad), np.float32)
        maskT = np.zeros((128, E_pad), np.float32)
        tl = trg[eids] - k * TPC
        slots = np.arange(ne)
        mask[slots % 128, (slots // 128) * 128 + tl] = 1.0
        maskT[tl, (slots // 128) * 128 + slots % 128] = 1.0
        # host gather of edge features (pure indexing) + transpose,
        # shipped as bf16 hi + bf16 residual for f32-level pe accuracy
        ef_rows = np.zeros((E_pad, C), np.float32)
        ef_rows[:ne] = np.asarray(ef[src[eids], trg[eids]], np.float32)
        arrT = ef_rows.T.reshape(2, 128, E_pad)
        hi = arrT.astype(NPBF)
        res = (arrT - hi.astype(np.float32)).astype(NPF8)
        eT = np.ascontiguousarray(hi.transpose(1, 0, 2))
        eTr = np.ascontiguousarray(res.transpose(1, 0, 2))
        # remapped rows for the chunk-pipelined table2 AllGather:
        # node n=(kk*128+t) lands at row (t//32)*256 + kk*32 + (t%32)
        kk = src_s // TPC
        t = src_s % TPC
        src2 = (t // 32) * 256 + kk * 32 + (t % 32)
        m = dict(common)
        m.update({
            "eT": eT, "eTr": eTr,
            "xT_loc": _sb3(np.ascontiguousarray(
                xf.T[:, k * NB_LOCAL:(k + 1) * NB_LOCAL]), NB_LOCAL),
            "isrc": _pack_idx(src_s),
            "isrc2": _pack_idx(src2),
            "mask": mask.astype(NPF8),
            "maskT": maskT.astype(NPF8),
        })
        in_maps.append(m)
    return in_maps, E_pad, tuple(need), n_chunks


# --------------------------------------------------------------------------
# device program
# --------------------------------------------------------------------------

def _build(E_pad: int, need: tuple, debug: bool = False):
    n_super = (E_pad + 511) // 512
    n_chunks = E_pad // 128
    assert len(need) == n_super
    nc = bacc.Bacc("TRN2", target_bir_lowering=False, debug=False,
                   num_devices=NC)

    xTf_in = nc.dram_tensor("xT_full", [128, 2, NB], BF16,
                            kind="ExternalInput")
    xTl_in = nc.dram_tensor("xT_loc", [128, 2, NB_LOCAL], BF16,
                            kind="ExternalInput")
    eT_in = nc.dram_tensor("eT", [128, 2, E_pad], BF16, kind="ExternalInput")
    eTr_in = nc.dram_tensor("eTr", [128, 2, E_pad], F8, kind="ExternalInput")
    isrc_in = nc.dram_tensor("isrc", [128, E_pad // 16], I16,
                             kind="ExternalInput")
    isrc2_in = nc.dram_tensor("isrc2", [128, E_pad // 16], I16,
                              kind="ExternalInput")
    mask_in = nc.dram_tensor("mask", [128, E_pad], F8, kind="ExternalInput")
    maskT_in = nc.dram_tensor("maskT", [128, E_pad], F8,
                              kind="ExternalInput")
    WOFF = {}
    off = 0
    for nm, inner in [("wab1", 264), ("wab2", 264), ("wn1hd", C),
                      ("wn2hd", C), ("we1hd", C), ("we2hd", C),
                      ("hselb1", B * H), ("hselb2", B * H),
                      ("ablk1", 2 * H), ("ablk2", 2 * H)]:
        WOFF[nm] = (off, inner)
        off += inner
    wpack_in = nc.dram_tensor("wpack", [128, 2, off], BF16,
                              kind="ExternalInput")
    ident_in = nc.dram_tensor("ident", [128, 128], BF16,
                              kind="ExternalInput")
    ident32_in = nc.dram_tensor("ident32", [B * H, B * H], F32,
                                kind="ExternalInput")
    y_out = nc.dram_tensor("y", [128, B * C], F32, kind="ExternalOutput")
    dbg = {}
    if debug:
        for nm, shape, dt in [("dbg_tbl", [N, ROW], BF16),
                              ("dbg_pe", [128, n_chunks, 2 * B * H], F32),
                              ("dbg_at", [128, 2 * B * H], F32),
                              ("dbg_x1", [128, B * C], F32),
                              ("dbg_den", [128, B * H], F32),
                              ("dbg_s4", [128, n_super, 4 * B * H], F32),
                              ("dbg_g", [128, 4, ROW], BF16)]:
            dbg[nm] = nc.dram_tensor(nm, shape, dt, kind="ExternalOutput")

    from contextlib import ExitStack
    with tile.TileContext(nc) as tc:
        with ExitStack() as ctx:
            const = ctx.enter_context(tc.tile_pool(name="const", bufs=1))
            sb = ctx.enter_context(tc.tile_pool(name="sb", bufs=1))
            small = ctx.enter_context(tc.tile_pool(name="small", bufs=3))
            gpool = ctx.enter_context(tc.tile_pool(name="gpool", bufs=5))
            ps_small = ctx.enter_context(
                tc.tile_pool(name="ps_small", bufs=3, space="PSUM"))
            ps_pat = ctx.enter_context(
                tc.tile_pool(name="ps_pat", bufs=2, space="PSUM"))
            ps_out = ctx.enter_context(
                tc.tile_pool(name="ps_out", bufs=1, space="PSUM"))
            ps_den = ctx.enter_context(
                tc.tile_pool(name="ps_den", bufs=1, space="PSUM"))
            dram = ctx.enter_context(tc.tile_pool(name="dram", bufs=1,
                                                  space="DRAM"))

            # ---- constants into SBUF (ordered: build-critical first)
            wpack_sb = const.tile([128, 2, wpack_in.shape[2]], BF16)
            nc.sync.dma_start(out=wpack_sb[:], in_=wpack_in[:])
            w_sb = {nm: wpack_sb[:, :, o:o + inner]
                    for nm, (o, inner) in WOFF.items()}
            xTf_sb = const.tile([128, 2, NB], BF16)
            for q in range(4):
                nc.sync.dma_start(out=xTf_sb[:, :, q * 1024:(q + 1) * 1024],
                                  in_=xTf_in[:, :, q * 1024:(q + 1) * 1024])
            xTl_sb = const.tile([128, 2, NB_LOCAL], BF16)
            nc.sync.dma_start(out=xTl_sb[:], in_=xTl_in[:])
            isrc_t = const.tile([128, E_pad // 16], I16)
            nc.sync.dma_start(out=isrc_t[:], in_=isrc_in[:])
            isrc2_t = const.tile([128, E_pad // 16], I16)
            nc.sync.dma_start(out=isrc2_t[:], in_=isrc2_in[:])
            ident = const.tile([128, 128], BF16)
            nc.sync.dma_start(out=ident[:], in_=ident_in[:])
            ident32 = const.tile([B * H, B * H], F32)
            nc.sync.dma_start(out=ident32[:], in_=ident32_in[:])
            # bulk inputs on the scalar HWDGE queue (parallel with sync)
            half = (n_chunks // 2) * 128
            eT_sb = const.tile([128, 2, E_pad], BF16)
            eTr_sb = const.tile([128, 2, E_pad], F8)
            maskT_sb = const.tile([128, E_pad], F8)
            mask_sb = const.tile([128, E_pad], F8)
            for lo, hi_ in ((0, half), (half, E_pad)):
                nc.scalar.dma_start(out=eT_sb[:, :, lo:hi_],
                                    in_=eT_in[:, :, lo:hi_])
                nc.scalar.dma_start(out=eTr_sb[:, :, lo:hi_],
                                    in_=eTr_in[:, :, lo:hi_])
                nc.scalar.dma_start(out=maskT_sb[:, lo:hi_],
                                    in_=maskT_in[:, lo:hi_])
                nc.scalar.dma_start(out=mask_sb[:, lo:hi_],
                                    in_=mask_in[:, lo:hi_])

            # ---- wesum_rep[c, (layer, b, h) hi | res] bf16 via on-device mm
            wesum_rep = const.tile([128, 2, 4 * B * H], BF16)
            for ct in range(2):
                pw = ps_small.tile([128, 2 * B * H], F32, space="PSUM",
                                   tag="ps", name="pw")
                for lj, (wehd, hs) in enumerate(
                        [("we1hd", "hselb1"), ("we2hd", "hselb2")]):
                    for kh in range(2):
                        nc.tensor.matmul(
                            out=pw[:, lj * B * H:(lj + 1) * B * H],
                            lhsT=w_sb[wehd][:, kh, ct * 128:(ct + 1) * 128],
                            rhs=w_sb[hs][:, kh, :],
                            start=(kh == 0), stop=(kh == 1))
                hi = 2 * B * H
                nc.scalar.copy(out=wesum_rep[:, ct, 0:hi], in_=pw[:])
                wtmp = small.tile([128, 2 * B * H], F32, tag="wtmp")
                nc.vector.tensor_copy(out=wtmp[:], in_=wesum_rep[:, ct, 0:hi])
                nc.vector.tensor_tensor(out=wtmp[:], in0=pw[:], in1=wtmp[:],
                                        op=mybir.AluOpType.subtract)
                nc.vector.tensor_copy(out=wesum_rep[:, ct, hi:2 * hi],
                                      in_=wtmp[:])

            # ---- per-layer projection rhs: [Wn cols (d,h) | Wn.T@ablk]
            # composed a-columns are written into the reserved pack slots
            def make_wab(wab_nm, wnhd, ablk):
                wab = w_sb[wab_nm]
                for ct in range(2):
                    pa = ps_small.tile([128, 2 * H], F32, space="PSUM",
                                       tag="ps", name="pcomp")
                    for kh in range(2):
                        nc.tensor.matmul(
                            out=pa[:],
                            lhsT=w_sb[wnhd][:, kh, ct * 128:(ct + 1) * 128],
                            rhs=w_sb[ablk][:, kh, :],
                            start=(kh == 0), stop=(kh == 1))
                    nc.scalar.copy(out=wab[:, ct, 256:264], in_=pa[:])
                return wab

            wab1 = make_wab("wab1", "wn1hd", "ablk1")
            wab2 = make_wab("wab2", "wn2hd", "ablk2")

            # ---- local a_tgt rhs (hi/res bf16 split) from a local lhsT
            def make_at(lhsT_sb, wab, tag):
                at_loc = small.tile([128, B * H], F32, tag="atl")
                for b in range(B):
                    pab = ps_small.tile([128, H], F32, space="PSUM",
                                        tag="ps", name="pab")
                    for ch in range(2):
                        lhsT_b = lhsT_sb[:, ch, :].rearrange(
                            "p (n b2) -> p b2 n", b2=B)[:, b, :]
                        nc.tensor.matmul(out=pab[:], lhsT=lhsT_b,
                                         rhs=wab[:, ch, 260:264],
                                         start=(ch == 0), stop=(ch == 1))
                    nc.vector.tensor_copy(out=at_loc[:, b * H:(b + 1) * H],
                                          in_=pab[:])
                at_rhs = sb.tile([128, 2 * B * H], BF16, tag=f"atr{tag}",
                                 name=f"atr{tag}")
                at_tmp = small.tile([128, B * H], F32, tag="att")
                nc.vector.tensor_copy(out=at_rhs[:, 0:B * H], in_=at_loc[:])
                nc.vector.tensor_copy(out=at_tmp[:], in_=at_rhs[:, 0:B * H])
                nc.vector.tensor_tensor(out=at_tmp[:], in0=at_loc[:],
                                        in1=at_tmp[:],
                                        op=mybir.AluOpType.subtract)
                nc.vector.tensor_copy(out=at_rhs[:, B * H:2 * B * H],
                                      in_=at_tmp[:])
                return at_rhs

            # ---- node-table build: rows [h (b,d,h) bf16 | a_src f32]
            def build_chunks(table, lhsT_sb, wab, chunks, row0):
                # each chunk covers 128 (node, b) rows = 32 nodes
                for t in chunks:
                    ph = ps_small.tile([128, 260], F32, space="PSUM",
                                       tag="ps", name="ph")
                    for ch in range(2):
                        nc.tensor.matmul(
                            out=ph[:],
                            lhsT=lhsT_sb[:, ch, t * 128:(t + 1) * 128],
                            rhs=wab[:, ch, 0:260],
                            start=(ch == 0), stop=(ch == 1))
                    sh = small.tile([128, BROW], BF16, tag="sh")
                    if t % 2 == 0:
                        nc.vector.tensor_copy(out=sh[:, 0:256],
                                              in_=ph[:, 0:256])
                        nc.scalar.copy(out=sh[:, 256:264].bitcast(F32),
                                       in_=ph[:, 256:260])
                    else:
                        nc.scalar.copy(out=sh[:, 0:256], in_=ph[:, 0:256])
                        nc.vector.tensor_copy(
                            out=sh[:, 256:264].bitcast(F32),
                            in_=ph[:, 256:260])
                    rows = slice(row0 + (t - chunks[0]) * 32,
                                 row0 + (t - chunks[0]) * 32 + 32)
                    nc.sync.dma_start(
                        out=table[rows, 0:B * BROW].rearrange(
                            "n (b o) -> n b o", b=B),
                        in_=sh[:])

            # every core builds the FULL layer-1 table locally: no collective
            # before the layer-2 AllGather, so cross-core launch jitter is
            # absorbed by local work instead of an early barrier.
            # pe chunks are interleaved with build chunks to keep PE fed
            # while build copies/DMAs drain.
            table1 = dram.tile([N, ROW], BF16, tag="tbl1", name="tbl1")
            pe_sb = sb.tile([128, n_chunks, 2 * B * H], F32)

            def pe_chunk(c):
                pp = ps_small.tile([128, 4 * B * H], F32, space="PSUM",
                                   tag="ps", name="pp")
                for i, (src_sb, ch) in enumerate(
                        [(eT_sb, 0), (eTr_sb, 0), (eT_sb, 1), (eTr_sb, 1)]):
                    nc.tensor.matmul(
                        out=pp[:],
                        lhsT=src_sb[:, ch, c * 128:(c + 1) * 128],
                        rhs=wesum_rep[:, ch, :],
                        start=(i == 0), stop=(i == 3))
                nc.scalar.copy(out=pe_sb[:, c, :], in_=pp[:, 0:2 * B * H])
                nc.vector.tensor_tensor(
                    out=pe_sb[:, c, :], in0=pe_sb[:, c, :],
                    in1=pp[:, 2 * B * H:4 * B * H],
                    op=mybir.AluOpType.add)

            at1 = make_at(xTl_sb, wab1, 1)

            # first collective dispatched late (post-jitter) to absorb
            # one-time CC init before the layer-2 AllGather
            wz = small.tile([16, 128], BF16, tag="wz")
            nc.vector.memset(wz[:], 0.0)
            warm_src = dram.tile([16, 128], BF16, tag="wsrc", name="wsrc")
            nc.sync.dma_start(out=warm_src[:], in_=wz[:])
            warm = dram.tile([128, 128], BF16, addr_space="Shared",
                             tag="warm", name="warm")
            nc.gpsimd.collective_compute(
                "AllGather", mybir.AluOpType.bypass,
                replica_groups=[list(range(NC))],
                ins=[warm_src.opt()], outs=[warm.opt()])
            warm2 = dram.tile([128, 128], BF16, addr_space="Shared",
                              tag="warm2", name="warm2")
            nc.gpsimd.collective_compute(
                "AllGather", mybir.AluOpType.bypass,
                replica_groups=[list(range(NC))],
                ins=[warm_src.opt()], outs=[warm2.opt()])

            # ---- edge loop for one layer (pipelined gathers)
            def edge_loop(table, at_rhs, layer, idx_t, pre_super=None,
                          row_bound=None):
                out_p = ps_out.tile([128, B * C], F32, space="PSUM",
                                    tag="out", name="out_p")
                den_p = ps_den.tile([128, B * H], F32, space="PSUM",
                                    tag="den", name="den_p")
                for s in range(n_super):
                    c0 = s * 4
                    nj = min(4, n_chunks - c0)
                    if pre_super is not None:
                        pre_super(s)
                    rb = N if row_bound is None else row_bound[s]
                    G = gpool.tile([128, nj, ROW], BF16, tag="G")
                    nc.gpsimd.dma_gather(
                        out_ap=G[:], in_ap=table[0:rb, :],
                        idxs_ap=idx_t[:, c0 * 8:(c0 + nj) * 8],
                        num_idxs=nj * 128, num_idxs_reg=nj * 128,
                        elem_size=ROW, single_packet=True)
                    pat4 = ps_pat.tile([128, nj, 2 * B * H], F32, space="PSUM",
                                       tag="pat", name="pat4")
                    for j in range(nj):
                        c = c0 + j
                        nc.tensor.matmul(
                            out=pat4[:, j, :],
                            lhsT=maskT_sb[:, c * 128:(c + 1) * 128],
                            rhs=at_rhs[:], start=True, stop=True)
                    s4 = small.tile([128, nj, B * H], F32, tag="s4")
                    g_as = G[:, :, 0:B * BROW].rearrange(
                        "p j (b o) -> p j b o", b=B)[:, :, :, 256:264].bitcast(
                        F32)
                    nc.vector.tensor_tensor(
                        out=s4[:].rearrange("p j (b h) -> p j b h", b=B),
                        in0=g_as,
                        in1=pe_sb[:, c0:c0 + nj,
                                  layer * B * H:(layer + 1) * B * H]
                            .rearrange("p j (b h) -> p j b h", b=B),
                        op=mybir.AluOpType.add)
                    nc.vector.tensor_tensor(
                        out=s4[:], in0=s4[:], in1=pat4[:, :, 0:B * H],
                        op=mybir.AluOpType.add)
                    nc.vector.tensor_tensor(
                        out=s4[:], in0=s4[:], in1=pat4[:, :, B * H:2 * B * H],
                        op=mybir.AluOpType.add)
                    if debug and layer == 0:
                        nc.sync.dma_start(
                            out=dbg["dbg_s4"][:, s, 0:nj * B * H],
                            in_=s4[:].rearrange("p a b2 -> p (a b2)"))
                        if s == 0:
                            nc.sync.dma_start(out=dbg["dbg_g"][:],
                                              in_=G[:])
                    t4 = small.tile([128, nj, B * H], F32, tag="t4")
                    nc.scalar.mul(out=t4[:], in_=s4[:], mul=0.2)
                    nc.vector.tensor_tensor(out=s4[:], in0=s4[:], in1=t4[:],
                                            op=mybir.AluOpType.max)
                    e4 = small.tile([128, nj, B * H], BF16, tag="e4")
                    nc.scalar.activation(
                        out=e4[:], in_=s4[:],
                        func=mybir.ActivationFunctionType.Exp)
                    for j in range(nj):
                        c = c0 + j
                        gq = G[:, j, 0:B * BROW].rearrange(
                            "p (b o) -> p b o", b=B)
                        gh = gq[:, :, 0:256].rearrange(
                            "p b (d h) -> p b d h", d=D)
                        nc.vector.tensor_tensor(
                            out=gh, in0=gh,
                            in1=e4[:, j, :].rearrange(
                                "p (b o h) -> p b o h", b=B, o=1)
                                .to_broadcast([128, B, D, H]),
                            op=mybir.AluOpType.mult)
                        mk = mask_sb[:, c * 128:(c + 1) * 128]
                        first, last = (c == 0), (c == n_chunks - 1)
                        nc.tensor.matmul(out=out_p[:, 0:512], lhsT=mk,
                                         rhs=gq[:, 0:2, 0:256],
                                         start=first, stop=last)
                        nc.tensor.matmul(out=out_p[:, 512:1024], lhsT=mk,
                                         rhs=gq[:, 2:4, 0:256],
                                         start=first, stop=last)
                        nc.tensor.matmul(out=den_p[:], lhsT=mk,
                                         rhs=e4[:, j, :],
                                         start=first, stop=last)
                dsb = small.tile([128, B * H], F32, tag="d")
                nc.vector.tensor_scalar_add(dsb[:], den_p[:], 1e-16)
                rec = small.tile([128, B * H], F32, tag="r")
                nc.vector.reciprocal(rec[:], dsb[:])
                if debug and layer == 0:
                    nc.sync.dma_start(out=dbg["dbg_den"][:], in_=dsb[:])
                # un-permute (b,d,h) -> (b,h,d) while applying 1/den
                xo = sb.tile([128, B * C], F32, tag=f"xo{layer}",
                             name=f"xo{layer}")
                nc.vector.tensor_tensor(
                    out=xo[:].rearrange("p (b h d) -> p b h d", b=B, h=H),
                    in0=out_p[:].rearrange("p (b d h) -> p b h d", b=B, d=D),
                    in1=rec[:].rearrange("p (b h o) -> p b h o", b=B, o=1)
                        .to_broadcast([128, B, H, D]),
                    op=mybir.AluOpType.mult)
                return xo

            if debug:
                nc.sync.dma_start(out=dbg["dbg_tbl"][:], in_=table1[:])
                nc.sync.dma_start(out=dbg["dbg_pe"][:], in_=pe_sb[:])
                at1f = small.tile([128, 2 * B * H], F32, tag="atf")
                nc.vector.tensor_copy(out=at1f[:], in_=at1[:])
                nc.sync.dma_start(out=dbg["dbg_at"][:], in_=at1f[:])

            state = {"built": 0, "pe": 0}

            def pre1(s):
                while state["built"] < need[s]:
                    t = state["built"]
                    build_chunks(table1, xTf_sb, wab1, [t], t * 32)
                    state["built"] += 1
                lim = min(n_chunks, (s + 1) * 4)
                while state["pe"] < lim:
                    pe_chunk(state["pe"])
                    state["pe"] += 1

            x1 = edge_loop(table1, at1, 0, isrc_t, pre_super=pre1,
                           row_bound=[nd * 32 for nd in need])
            if debug:
                nc.sync.dma_start(out=dbg["dbg_x1"][:], in_=x1[:])

            # ---- layer boundary: x1 -> x1T (bf16) -> local table2 + AG
            x1b = sb.tile([128, B * C], BF16)
            nc.vector.tensor_copy(out=x1b[:], in_=x1[:])
            x1T = sb.tile([128, 2, NB_LOCAL], BF16)
            for b in range(B):
                for ch in range(2):
                    pt = ps_pat.tile([128, 128], BF16, space="PSUM",
                                     tag="pat", name="pt")
                    nc.tensor.transpose(
                        out=pt[:],
                        in_=x1b[:, b * C + ch * 128: b * C + (ch + 1) * 128],
                        identity=ident[:])
                    eng = nc.scalar if (b + ch) % 2 == 0 else nc.vector
                    if eng is nc.scalar:
                        eng.copy(
                            out=x1T[:, ch, :].rearrange(
                                "p (n b2) -> p n b2", b2=B)[:, :, b],
                            in_=pt[:])
                    else:
                        eng.tensor_copy(
                            out=x1T[:, ch, :].rearrange(
                                "p (n b2) -> p n b2", b2=B)[:, :, b],
                            in_=pt[:])

            ag_in = dram.tile([TPC, ROW], BF16, tag="agin", name="agin")
            table2 = dram.tile([N, ROW], BF16, addr_space="Shared",
                               tag="tbl2", name="tbl2")
            build_chunks(ag_in, x1T, wab2, list(range(4)), 0)
            at2 = make_at(x1T, wab2, 2)
            nc.gpsimd.collective_compute(
                "AllGather", mybir.AluOpType.bypass,
                replica_groups=[list(range(NC))],
                ins=[ag_in.opt()], outs=[table2.opt()])

            x2 = edge_loop(table2, at2, 1, isrc_t)
            nc.sync.dma_start(out=y_out[:], in_=x2[:])

    nc.compile()
    return nc


_CACHE: dict = {}


def _get_program(E_pad: int, need: tuple, debug: bool = False):
    key = (E_pad, need, debug)
    if key not in _CACHE:
        _CACHE[key] = _build(E_pad, need, debug)
    return _CACHE[key]


def kernel(debug=False, trace=False, **inputs):
    in_maps, E_pad, need, n_chunks = _prep(**inputs)
    nc = _get_program(E_pad, need, debug)
    res = run_bass_kernel_spmd(nc, in_maps, core_ids=list(range(NC)),
                               trace=trace)
    y = np.concatenate([res.results[k]["y"] for k in range(NC)], axis=0)
    out = y.reshape(N, B, C)
    if debug or trace:
        return out, res
    return out


# revision 81
# speedup vs baseline: 1.0652x; 1.0652x over previous
"""Trainium2 Bass kernel for a 2-layer GAT (nn_GAT_70909910057105).

Strategy (8 NeuronCores, SPMD):
  - Core k owns target nodes [128k, 128k+128). Edges bucketed by trg//128 on
    the host (integer-only preprocessing + dtype casts).
  - Every core builds the FULL layer-1 node table locally (bf16 matmuls from
    a replicated bf16 xT) -- no collective before layer 1. Layer 2 rebuilds
    the table from the core-local x1 shard and AllGathers it.
  - A bf16 DRAM node table holds per-node rows
    [h bf16 x1024 (b,d,h layout) | a_src f32 x16 (bitcast) | pad] (1152 bf16).
    Per-edge source rows are fetched with pipelined dma_gather
    (prepare_only + trigger_dma) so gpsimd only does descriptor generation.
  - Edge features are gathered ON HOST (pure integer indexing + bf16 cast)
    and shipped pre-transposed, so pe = eT.T @ wesum is a plain bf16 matmul.
  - segment_sum is a PSUM-accumulated bf16 matmul with host-built one-hot
    masks; per-edge target alphas come from maskT.T @ [at_hi | at_res].
  - Table h layout (b,d,h) makes the per-edge exp-score broadcast multiply
    hit the DVE 2x fast mode (innermost dim packed, all operands bf16).
"""
import sys

for _p in ("/opt/trn_rl_repo", "/root/.axon_site/_ro/trn_rl_repo"):
    if _p not in sys.path:
        sys.path.insert(0, _p)

import numpy as np
import ml_dtypes
import concourse.bass as bass
import concourse.bacc as bacc
import concourse.tile as tile
from concourse import mybir
from concourse.bass_utils import run_bass_kernel_spmd
from concourse.masks import make_identity

F32 = mybir.dt.float32
BF16 = mybir.dt.bfloat16
I16 = mybir.dt.int16
NPBF = ml_dtypes.bfloat16
NPF8 = ml_dtypes.float8_e4m3
F8 = mybir.dt.float8e4

USE_PREP = False  # pipelined prepare_only gathers (see edge_loop)

N, B, C, H, D = 1024, 4, 256, 4, 64
E = 32768
NC = 8
TPC = N // NC           # target nodes per core = 128
# row: 4 x [256 h (d,h layout) | 8 (4 f32 a_src, bitcast)] then pad -> 1152
ROW = 1152
BROW = 264              # bf16 elems per b-block: 256 h + 8 a_src
NB = N * B              # 4096 (node, batch) rows
NB_LOCAL = TPC * B      # 512 local (node, batch) rows

# column permutation: j = d*H + h  ->  c = h*D + d  (h block layout (d, h))
_DH_PERM = np.array([(j % H) * D + j // H for j in range(C)], dtype=np.int64)


# --------------------------------------------------------------------------
# host-side preprocessing (integer indexing / layout / dtype casts only)
# --------------------------------------------------------------------------

def _pack_idx(vals: np.ndarray) -> np.ndarray:
    n = vals.shape[0]
    assert n % 16 == 0
    blk = vals.astype(np.int16).reshape(n // 16, 16).T
    return np.ascontiguousarray(np.tile(blk, (8, 1)))


def _sb3(w: np.ndarray, inner: int, dt=NPBF) -> np.ndarray:
    return np.ascontiguousarray(
        w.reshape(2, 128, inner).transpose(1, 0, 2).astype(dt))


def _prep(x, edge_features, src_idx, trg_idx,
          Wn1, We1, a_src1, a_tgt1, a_edge1,
          Wn2, We2, a_src2, a_tgt2, a_edge2):
    src = np.asarray(src_idx).astype(np.int64)
    trg = np.asarray(trg_idx).astype(np.int64)
    x = np.asarray(x, dtype=np.float32)

    per_core = []
    emax = 0
    for k in range(NC):
        eids = np.nonzero((trg // TPC) == k)[0]
        eids = eids[np.argsort(src[eids], kind="stable")]
        per_core.append(eids)
        emax = max(emax, len(eids))
    E_pad = ((emax + 127) // 128) * 128
    n_super = (E_pad + 511) // 512
    n_chunks = E_pad // 128

    xf = x.reshape(NB, C)

    def build_w(Wn):
        # Wn.T with columns permuted to (d,h) order -> [C, 256]
        return _sb3(np.ascontiguousarray(
            np.asarray(Wn, np.float32).T[:, _DH_PERM]), C)

    def build_ablk(a_s, a_t):
        # block-diagonal [a_src | a_tgt] -> [C, 2H]
        m = np.zeros((C, 2 * H), np.float32)
        a_s = np.asarray(a_s, np.float32)
        a_t = np.asarray(a_t, np.float32)
        for h in range(H):
            m[h * D:(h + 1) * D, h] = a_s[h]
            m[h * D:(h + 1) * D, H + h] = a_t[h]
        return _sb3(m, 2 * H)

    def build_hselb(a_e):
        # b-replicated head selector: [C, 16], col = b*H + h
        m = np.zeros((C, B * H), np.float32)
        a_e = np.asarray(a_e, np.float32)
        for h in range(H):
            for b in range(B):
                m[h * D:(h + 1) * D, b * H + h] = a_e[h]
        return _sb3(m, B * H)

    zpad8 = np.zeros((128, 2, 8), NPBF)
    wpack = np.concatenate([
        build_w(Wn1), zpad8, build_w(Wn2), zpad8,
        _sb3(np.asarray(Wn1, np.float32), C),
        _sb3(np.asarray(Wn2, np.float32), C),
        _sb3(np.asarray(We1, np.float32), C),
        _sb3(np.asarray(We2, np.float32), C),
        build_hselb(a_edge1), build_hselb(a_edge2),
        build_ablk(a_src1, a_tgt1), build_ablk(a_src2, a_tgt2),
    ], axis=2)
    common = {
        "wpack": np.ascontiguousarray(wpack),
        "ident": np.eye(128, dtype=NPBF),
        "ident32": np.eye(B * H, dtype=np.float32),
        "xT_full": _sb3(np.ascontiguousarray(xf.T), NB),
    }

    # per-superstep table prefix needed by the (src-sorted) gathers
    need = [1] * n_super
    ef = edge_features  # only sliced rows are materialized below
    in_maps = []
    for k in range(NC):
        eids = per_core[k]
        ne = len(eids)
        src_s = np.zeros(E_pad, np.int64)
        src_s[:ne] = src[eids]
        for s in range(n_super):
            hi = min((s + 1) * 512, E_pad)
            rmax = int(src_s[:hi].max()) + 1
            need[s] = max(need[s], (rmax + 31) // 32)
        mask = np.zeros((128, E_pad), np.float32)
        maskT = np.zeros((128, E_pad), np.float32)
        tl = trg[eids] - k * TPC
        slots = np.arange(ne)
        mask[slots % 128, (slots // 128) * 128 + tl] = 1.0
        maskT[tl, (slots // 128) * 128 + slots % 128] = 1.0
        # host gather of edge features (pure indexing) + transpose,
        # shipped as bf16 hi + bf16 residual for f32-level pe accuracy
        ef_rows = np.zeros((E_pad, C), np.float32)
        ef_rows[:ne] = np.asarray(ef[src[eids], trg[eids]], np.float32)
        arrT = ef_rows.T.reshape(2, 128, E_pad)
        hi = arrT.astype(NPBF)
        res = (arrT - hi.astype(np.float32)).astype(NPF8)
        eT = np.ascontiguousarray(hi.transpose(1, 0, 2))
        eTr = np.ascontiguousarray(res.transpose(1, 0, 2))
        # remapped rows for the chunk-pipelined table2 AllGather:
        # node n=(kk*128+t) lands at row (t//32)*256 + kk*32 + (t%32)
        kk = src_s // TPC
        t = src_s % TPC
        src2 = (t // 32) * 256 + kk * 32 + (t % 32)
        m = dict(common)
        m.update({
            "eT": eT, "eTr": eTr,
            "xT_loc": _sb3(np.ascontiguousarray(
                xf.T[:, k * NB_LOCAL:(k + 1) * NB_LOCAL]), NB_LOCAL),
            "isrc": _pack_idx(src_s),
            "isrc2": _pack_idx(src2),
            "mask": mask.astype(NPF8),
            "maskT": maskT.astype(NPF8),
        })
        in_maps.append(m)
    return in_maps, E_pad, tuple(need), n_chunks


# --------------------------------------------------------------------------
# device program
# --------------------------------------------------------------------------

def _build(E_pad: int, need: tuple, debug: bool = False):
    n_super = (E_pad + 511) // 512
    n_chunks = E_pad // 128
    assert len(need) == n_super
    nc = bacc.Bacc("TRN2", target_bir_lowering=False, debug=False,
                   num_devices=NC)

    xTf_in = nc.dram_tensor("xT_full", [128, 2, NB], BF16,
                            kind="ExternalInput")
    xTl_in = nc.dram_tensor("xT_loc", [128, 2, NB_LOCAL], BF16,
                            kind="ExternalInput")
    eT_in = nc.dram_tensor("eT", [128, 2, E_pad], BF16, kind="ExternalInput")
    eTr_in = nc.dram_tensor("eTr", [128, 2, E_pad], F8, kind="ExternalInput")
    isrc_in = nc.dram_tensor("isrc", [128, E_pad // 16], I16,
                             kind="ExternalInput")
    isrc2_in = nc.dram_tensor("isrc2", [128, E_pad // 16], I16,
                              kind="ExternalInput")
    mask_in = nc.dram_tensor("mask", [128, E_pad], F8, kind="ExternalInput")
    maskT_in = nc.dram_tensor("maskT", [128, E_pad], F8,
                              kind="ExternalInput")
    WOFF = {}
    off = 0
    for nm, inner in [("wab1", 264), ("wab2", 264), ("wn1hd", C),
                      ("wn2hd", C), ("we1hd", C), ("we2hd", C),
                      ("hselb1", B * H), ("hselb2", B * H),
                      ("ablk1", 2 * H), ("ablk2", 2 * H)]:
        WOFF[nm] = (off, inner)
        off += inner
    wpack_in = nc.dram_tensor("wpack", [128, 2, off], BF16,
                              kind="ExternalInput")
    ident_in = nc.dram_tensor("ident", [128, 128], BF16,
                              kind="ExternalInput")
    ident32_in = nc.dram_tensor("ident32", [B * H, B * H], F32,
                                kind="ExternalInput")
    y_out = nc.dram_tensor("y", [128, B * C], F32, kind="ExternalOutput")
    dbg = {}
    if debug:
        for nm, shape, dt in [("dbg_tbl", [N, ROW], BF16),
                              ("dbg_pe", [128, n_chunks, 2 * B * H], F32),
                              ("dbg_at", [128, 2 * B * H], F32),
                              ("dbg_x1", [128, B * C], F32),
                              ("dbg_den", [128, B * H], F32),
                              ("dbg_s4", [128, n_super, 4 * B * H], F32),
                              ("dbg_g", [128, 4, ROW], BF16)]:
            dbg[nm] = nc.dram_tensor(nm, shape, dt, kind="ExternalOutput")

    from contextlib import ExitStack
    with tile.TileContext(nc) as tc:
        with ExitStack() as ctx:
            const = ctx.enter_context(tc.tile_pool(name="const", bufs=1))
            sb = ctx.enter_context(tc.tile_pool(name="sb", bufs=1))
            small = ctx.enter_context(tc.tile_pool(name="small", bufs=3))
            gpool = ctx.enter_context(tc.tile_pool(name="gpool", bufs=5))
            ps_small = ctx.enter_context(
                tc.tile_pool(name="ps_small", bufs=3, space="PSUM"))
            ps_pat = ctx.enter_context(
                tc.tile_pool(name="ps_pat", bufs=2, space="PSUM"))
            ps_out = ctx.enter_context(
                tc.tile_pool(name="ps_out", bufs=1, space="PSUM"))
            ps_den = ctx.enter_context(
                tc.tile_pool(name="ps_den", bufs=1, space="PSUM"))
            dram = ctx.enter_context(tc.tile_pool(name="dram", bufs=1,
                                                  space="DRAM"))

            # ---- constants into SBUF (ordered: build-critical first)
            wpack_sb = const.tile([128, 2, wpack_in.shape[2]], BF16)
            nc.sync.dma_start(out=wpack_sb[:], in_=wpack_in[:])
            w_sb = {nm: wpack_sb[:, :, o:o + inner]
                    for nm, (o, inner) in WOFF.items()}
            xTf_sb = const.tile([128, 2, NB], BF16)
            for q in range(4):
                nc.sync.dma_start(out=xTf_sb[:, :, q * 1024:(q + 1) * 1024],
                                  in_=xTf_in[:, :, q * 1024:(q + 1) * 1024])
            xTl_sb = const.tile([128, 2, NB_LOCAL], BF16)
            nc.sync.dma_start(out=xTl_sb[:], in_=xTl_in[:])
            isrc_t = const.tile([128, E_pad // 16], I16)
            nc.sync.dma_start(out=isrc_t[:], in_=isrc_in[:])
            isrc2_t = const.tile([128, E_pad // 16], I16)
            nc.sync.dma_start(out=isrc2_t[:], in_=isrc2_in[:])
            ident = const.tile([128, 128], BF16)
            nc.sync.dma_start(out=ident[:], in_=ident_in[:])
            ident32 = const.tile([B * H, B * H], F32)
            nc.sync.dma_start(out=ident32[:], in_=ident32_in[:])
            # bulk inputs on the scalar HWDGE queue (parallel with sync)
            half = (n_chunks // 2) * 128
            eT_sb = const.tile([128, 2, E_pad], BF16)
            eTr_sb = const.tile([128, 2, E_pad], F8)
            maskT_sb = const.tile([128, E_pad], F8)
            mask_sb = const.tile([128, E_pad], F8)
            for lo, hi_ in ((0, half), (half, E_pad)):
                nc.scalar.dma_start(out=eT_sb[:, :, lo:hi_],
                                    in_=eT_in[:, :, lo:hi_])
                nc.scalar.dma_start(out=eTr_sb[:, :, lo:hi_],
                                    in_=eTr_in[:, :, lo:hi_])
                nc.scalar.dma_start(out=maskT_sb[:, lo:hi_],
                                    in_=maskT_in[:, lo:hi_])
                nc.scalar.dma_start(out=mask_sb[:, lo:hi_],
                                    in_=mask_in[:, lo:hi_])

            # ---- wesum_rep[c, (layer, b, h) hi | res] bf16 via on-device mm
            wesum_rep = const.tile([128, 2, 4 * B * H], BF16)
            for ct in range(2):
                pw = ps_small.tile([128, 2 * B * H], F32, space="PSUM",
                                   tag="ps", name="pw")
                for lj, (wehd, hs) in enumerate(
                        [("we1hd", "hselb1"), ("we2hd", "hselb2")]):
                    for kh in range(2):
                        nc.tensor.matmul(
                            out=pw[:, lj * B * H:(lj + 1) * B * H],
                            lhsT=w_sb[wehd][:, kh, ct * 128:(ct + 1) * 128],
                            rhs=w_sb[hs][:, kh, :],
                            start=(kh == 0), stop=(kh == 1))
                hi = 2 * B * H
                nc.scalar.copy(out=wesum_rep[:, ct, 0:hi], in_=pw[:])
                wtmp = small.tile([128, 2 * B * H], F32, tag="wtmp")
                nc.vector.tensor_copy(out=wtmp[:], in_=wesum_rep[:, ct, 0:hi])
                nc.vector.tensor_tensor(out=wtmp[:], in0=pw[:], in1=wtmp[:],
                                        op=mybir.AluOpType.subtract)
                nc.vector.tensor_copy(out=wesum_rep[:, ct, hi:2 * hi],
                                      in_=wtmp[:])

            # ---- per-layer projection rhs: [Wn cols (d,h) | Wn.T@ablk]
            # composed a-columns are written into the reserved pack slots
            def make_wab(wab_nm, wnhd, ablk):
                wab = w_sb[wab_nm]
                for ct in range(2):
                    pa = ps_small.tile([128, 2 * H], F32, space="PSUM",
                                       tag="ps", name="pcomp")
                    for kh in range(2):
                        nc.tensor.matmul(
                            out=pa[:],
                            lhsT=w_sb[wnhd][:, kh, ct * 128:(ct + 1) * 128],
                            rhs=w_sb[ablk][:, kh, :],
                            start=(kh == 0), stop=(kh == 1))
                    nc.scalar.copy(out=wab[:, ct, 256:264], in_=pa[:])
                return wab

            wab1 = make_wab("wab1", "wn1hd", "ablk1")
            wab2 = make_wab("wab2", "wn2hd", "ablk2")

            # ---- local a_tgt rhs (hi/res bf16 split) from a local lhsT
            def make_at(lhsT_sb, wab, tag):
                at_loc = small.tile([128, B * H], F32, tag="atl")
                for b in range(B):
                    pab = ps_small.tile([128, H], F32, space="PSUM",
                                        tag="ps", name="pab")
                    for ch in range(2):
                        lhsT_b = lhsT_sb[:, ch, :].rearrange(
                            "p (n b2) -> p b2 n", b2=B)[:, b, :]
                        nc.tensor.matmul(out=pab[:], lhsT=lhsT_b,
                                         rhs=wab[:, ch, 260:264],
                                         start=(ch == 0), stop=(ch == 1))
                    nc.vector.tensor_copy(out=at_loc[:, b * H:(b + 1) * H],
                                          in_=pab[:])
                at_rhs = sb.tile([128, 2 * B * H], BF16, tag=f"atr{tag}",
                                 name=f"atr{tag}")
                at_tmp = small.tile([128, B * H], F32, tag="att")
                nc.vector.tensor_copy(out=at_rhs[:, 0:B * H], in_=at_loc[:])
                nc.vector.tensor_copy(out=at_tmp[:], in_=at_rhs[:, 0:B * H])
                nc.vector.tensor_tensor(out=at_tmp[:], in0=at_loc[:],
                                        in1=at_tmp[:],
                                        op=mybir.AluOpType.subtract)
                nc.vector.tensor_copy(out=at_rhs[:, B * H:2 * B * H],
                                      in_=at_tmp[:])
                return at_rhs

            # ---- node-table build: rows [h (b,d,h) bf16 | a_src f32]
            def build_chunks(table, lhsT_sb, wab, chunks, row0):
                # each chunk covers 128 (node, b) rows = 32 nodes
                for t in chunks:
                    ph = ps_small.tile([128, 260], F32, space="PSUM",
                                       tag="ps", name="ph")
                    for ch in range(2):
                        nc.tensor.matmul(
                            out=ph[:],
                            lhsT=lhsT_sb[:, ch, t * 128:(t + 1) * 128],
                            rhs=wab[:, ch, 0:260],
                            start=(ch == 0), stop=(ch == 1))
                    sh = small.tile([128, BROW], BF16, tag="sh")
                    nc.vector.tensor_copy(out=sh[:, 0:256], in_=ph[:, 0:256])
                    nc.scalar.copy(out=sh[:, 256:264].bitcast(F32),
                                   in_=ph[:, 256:260])
                    rows = slice(row0 + (t - chunks[0]) * 32,
                                 row0 + (t - chunks[0]) * 32 + 32)
                    nc.sync.dma_start(
                        out=table[rows, 0:B * BROW].rearrange(
                            "n (b o) -> n b o", b=B),
                        in_=sh[:])

            # every core builds the FULL layer-1 table locally: no collective
            # before the layer-2 AllGather, so cross-core launch jitter is
            # absorbed by local work instead of an early barrier.
            # pe chunks are interleaved with build chunks to keep PE fed
            # while build copies/DMAs drain.
            table1 = dram.tile([N, ROW], BF16, tag="tbl1", name="tbl1")
            pe_sb = sb.tile([128, n_chunks, 2 * B * H], F32)

            def pe_chunk(c):
                pp = ps_small.tile([128, 4 * B * H], F32, space="PSUM",
                                   tag="ps", name="pp")
                for i, (src_sb, ch) in enumerate(
                        [(eT_sb, 0), (eTr_sb, 0), (eT_sb, 1), (eTr_sb, 1)]):
                    nc.tensor.matmul(
                        out=pp[:],
                        lhsT=src_sb[:, ch, c * 128:(c + 1) * 128],
                        rhs=wesum_rep[:, ch, :],
                        start=(i == 0), stop=(i == 3))
                nc.scalar.copy(out=pe_sb[:, c, :], in_=pp[:, 0:2 * B * H])
                nc.vector.tensor_tensor(
                    out=pe_sb[:, c, :], in0=pe_sb[:, c, :],
                    in1=pp[:, 2 * B * H:4 * B * H],
                    op=mybir.AluOpType.add)

            at1 = make_at(xTl_sb, wab1, 1)

            # first collective dispatched late (post-jitter) to absorb
            # one-time CC init before the layer-2 AllGather
            wz = small.tile([16, 128], BF16, tag="wz")
            nc.vector.memset(wz[:], 0.0)
            warm_src = dram.tile([16, 128], BF16, tag="wsrc", name="wsrc")
            nc.sync.dma_start(out=warm_src[:], in_=wz[:])
            warm = dram.tile([128, 128], BF16, addr_space="Shared",
                             tag="warm", name="warm")
            nc.gpsimd.collective_compute(
                "AllGather", mybir.AluOpType.bypass,
                replica_groups=[list(range(NC))],
                ins=[warm_src.opt()], outs=[warm.opt()])
            warm2 = dram.tile([128, 128], BF16, addr_space="Shared",
                              tag="warm2", name="warm2")
            nc.gpsimd.collective_compute(
                "AllGather", mybir.AluOpType.bypass,
                replica_groups=[list(range(NC))],
                ins=[warm_src.opt()], outs=[warm2.opt()])

            # ---- edge loop for one layer (pipelined gathers)
            def edge_loop(table, at_rhs, layer, idx_t, pre_super=None,
                          row_bound=None):
                out_p = ps_out.tile([128, B * C], F32, space="PSUM",
                                    tag="out", name="out_p")
                den_p = ps_den.tile([128, B * H], F32, space="PSUM",
                                    tag="den", name="den_p")
                for s in range(n_super):
                    c0 = s * 4
                    nj = min(4, n_chunks - c0)
                    if pre_super is not None:
                        pre_super(s)
                    rb = N if row_bound is None else row_bound[s]
                    G = gpool.tile([128, nj, ROW], BF16, tag="G")
                    nc.gpsimd.dma_gather(
                        out_ap=G[:], in_ap=table[0:rb, :],
                        idxs_ap=idx_t[:, c0 * 8:(c0 + nj) * 8],
                        num_idxs=nj * 128, num_idxs_reg=nj * 128,
                        elem_size=ROW, single_packet=True)
                    pat4 = ps_pat.tile([128, nj, 2 * B * H], F32, space="PSUM",
                                       tag="pat", name="pat4")
                    for j in range(nj):
                        c = c0 + j
                        nc.tensor.matmul(
                            out=pat4[:, j, :],
                            lhsT=maskT_sb[:, c * 128:(c + 1) * 128],
                            rhs=at_rhs[:], start=True, stop=True)
                    s4 = small.tile([128, nj, B * H], F32, tag="s4")
                    g_as = G[:, :, 0:B * BROW].rearrange(
                        "p j (b o) -> p j b o", b=B)[:, :, :, 256:264].bitcast(
                        F32)
                    nc.vector.tensor_tensor(
                        out=s4[:].rearrange("p j (b h) -> p j b h", b=B),
                        in0=g_as,
                        in1=pe_sb[:, c0:c0 + nj,
                                  layer * B * H:(layer + 1) * B * H]
                            .rearrange("p j (b h) -> p j b h", b=B),
                        op=mybir.AluOpType.add)
                    nc.vector.tensor_tensor(
                        out=s4[:], in0=s4[:], in1=pat4[:, :, 0:B * H],
                        op=mybir.AluOpType.add)
                    nc.vector.tensor_tensor(
                        out=s4[:], in0=s4[:], in1=pat4[:, :, B * H:2 * B * H],
                        op=mybir.AluOpType.add)
                    if debug and layer == 0:
                        nc.sync.dma_start(
                            out=dbg["dbg_s4"][:, s, 0:nj * B * H],
                            in_=s4[:].rearrange("p a b2 -> p (a b2)"))
                        if s == 0:
                            nc.sync.dma_start(out=dbg["dbg_g"][:],
                                              in_=G[:])
                    t4 = small.tile([128, nj, B * H], F32, tag="t4")
                    nc.scalar.mul(out=t4[:], in_=s4[:], mul=0.2)
                    nc.vector.tensor_tensor(out=s4[:], in0=s4[:], in1=t4[:],
                                            op=mybir.AluOpType.max)
                    e4 = small.tile([128, nj, B * H], BF16, tag="e4")
                    nc.scalar.activation(
                        out=e4[:], in_=s4[:],
                        func=mybir.ActivationFunctionType.Exp)
                    for j in range(nj):
                        c = c0 + j
                        gq = G[:, j, 0:B * BROW].rearrange(
                            "p (b o) -> p b o", b=B)
                        gh = gq[:, :, 0:256].rearrange(
                            "p b (d h) -> p b d h", d=D)
                        nc.vector.tensor_tensor(
                            out=gh, in0=gh,
                            in1=e4[:, j, :].rearrange(
                                "p (b o h) -> p b o h", b=B, o=1)
                                .to_broadcast([128, B, D, H]),
                            op=mybir.AluOpType.mult)
                        mk = mask_sb[:, c * 128:(c + 1) * 128]
                        first, last = (c == 0), (c == n_chunks - 1)
                        nc.tensor.matmul(out=out_p[:, 0:512], lhsT=mk,
                                         rhs=gq[:, 0:2, 0:256],
                                         start=first, stop=last)
                        nc.tensor.matmul(out=out_p[:, 512:1024], lhsT=mk,
                                         rhs=gq[:, 2:4, 0:256],
                                         start=first, stop=last)
                        nc.tensor.matmul(out=den_p[:], lhsT=mk,
                                         rhs=e4[:, j, :],
                                         start=first, stop=last)
                dsb = small.tile([128, B * H], F32, tag="d")
                nc.vector.tensor_scalar_add(dsb[:], den_p[:], 1e-16)
                rec = small.tile([128, B * H], F32, tag="r")
                nc.vector.reciprocal(rec[:], dsb[:])
                if debug and layer == 0:
                    nc.sync.dma_start(out=dbg["dbg_den"][:], in_=dsb[:])
                # un-permute (b,d,h) -> (b,h,d) while applying 1/den
                xo = sb.tile([128, B * C], F32, tag=f"xo{layer}",
                             name=f"xo{layer}")
                nc.vector.tensor_tensor(
                    out=xo[:].rearrange("p (b h d) -> p b h d", b=B, h=H),
                    in0=out_p[:].rearrange("p (b d h) -> p b h d", b=B, d=D),
                    in1=rec[:].rearrange("p (b h o) -> p b h o", b=B, o=1)
                        .to_broadcast([128, B, H, D]),
                    op=mybir.AluOpType.mult)
                return xo

            if debug:
                nc.sync.dma_start(out=dbg["dbg_tbl"][:], in_=table1[:])
                nc.sync.dma_start(out=dbg["dbg_pe"][:], in_=pe_sb[:])
                at1f = small.tile([128, 2 * B * H], F32, tag="atf")
                nc.vector.tensor_copy(out=at1f[:], in_=at1[:])
                nc.sync.dma_start(out=dbg["dbg_at"][:], in_=at1f[:])

            state = {"built": 0, "pe": 0}

            def pre1(s):
                while state["built"] < need[s]:
                    t = state["built"]
                    build_chunks(table1, xTf_sb, wab1, [t], t * 32)
                    state["built"] += 1
                lim = min(n_chunks, (s + 1) * 4)
                while state["pe"] < lim:
                    pe_chunk(state["pe"])
                    state["pe"] += 1

            x1 = edge_loop(table1, at1, 0, isrc_t, pre_super=pre1,
                           row_bound=[nd * 32 for nd in need])
            if debug:
                nc.sync.dma_start(out=dbg["dbg_x1"][:], in_=x1[:])

            # ---- layer boundary: x1 -> x1T (bf16) -> local table2 + AG
            x1b = sb.tile([128, B * C], BF16)
            nc.vector.tensor_copy(out=x1b[:], in_=x1[:])
            x1T = sb.tile([128, 2, NB_LOCAL], BF16)
            for b in range(B):
                for ch in range(2):
                    pt = ps_pat.tile([128, 128], BF16, space="PSUM",
                                     tag="pat", name="pt")
                    nc.tensor.transpose(
                        out=pt[:],
                        in_=x1b[:, b * C + ch * 128: b * C + (ch + 1) * 128],
                        identity=ident[:])
                    eng = nc.scalar if (b + ch) % 2 == 0 else nc.vector
                    if eng is nc.scalar:
                        eng.copy(
                            out=x1T[:, ch, :].rearrange(
                                "p (n b2) -> p n b2", b2=B)[:, :, b],
                            in_=pt[:])
                    else:
                        eng.tensor_copy(
                            out=x1T[:, ch, :].rearrange(
                                "p (n b2) -> p n b2", b2=B)[:, :, b],
                            in_=pt[:])

            ag_in = dram.tile([TPC, ROW], BF16, tag="agin", name="agin")
            table2 = dram.tile([N, ROW], BF16, addr_space="Shared",
                               tag="tbl2", name="tbl2")
            build_chunks(ag_in, x1T, wab2, list(range(4)), 0)
            at2 = make_at(x1T, wab2, 2)
            nc.gpsimd.collective_compute(
                "AllGather", mybir.AluOpType.bypass,
                replica_groups=[list(range(NC))],
                ins=[ag_in.opt()], outs=[table2.opt()])

            x2 = edge_loop(table2, at2, 1, isrc_t)
            nc.sync.dma_start(out=y_out[:], in_=x2[:])

    nc.compile()
    return nc


_CACHE: dict = {}


def _get_program(E_pad: int, need: tuple, debug: bool = False):
    key = (E_pad, need, debug)
    if key not in _CACHE:
        _CACHE[key] = _build(E_pad, need, debug)
    return _CACHE[key]


def kernel(debug=False, trace=False, **inputs):
    in_maps, E_pad, need, n_chunks = _prep(**inputs)
    nc = _get_program(E_pad, need, debug)
    res = run_bass_kernel_spmd(nc, in_maps, core_ids=list(range(NC)),
                               trace=trace)
    y = np.concatenate([res.results[k]["y"] for k in range(NC)], axis=0)
    out = y.reshape(N, B, C)
    if debug or trace:
        return out, res
    return out
